# revision 2
# baseline (speedup 1.0000x reference)
"""Trainium2 Bass kernel for nn_EntropyLM — v2 (see kernel.py for v1).

Changes vs v1 (262.5us):
  * V-ones: per-head ones column rides inside vN8 (layout [128, NT, 4, HD+1])
    so the softmax denominator accumulates in the same PSUM group as PV —
    kills 256 denominator matmuls + ldweights per core.
  * LN2 decoupled from the critical path: z is stored UNNORMALIZED
    ((res - mean)/16 in fp16); 1/sqrt(var) is applied per-token at the st7
    PSUM evacuation (tokens are partitions there) fused with the bias via
    scalar_tensor_tensor.  The sqrt is ONE batched [128,8] Act op per chunk
    at an exp-era boundary; same batching for LN1's iv.
  * Act table eras: sqrt(LN1 c0,c1) -> gelu(c0,c1) -> exp(c0) -> sqrt(LN2 c0)
    -> exp(c1) -> sqrt(LN2 c1): 6 table loads (was 13).
  * Wide 1024-col PSUM evacuations for q/k/v and w2; Pool (no PSUM port)
    takes the SBUF->SBUF work (LN1 apply, mix8 cast, otc8 cast); Act and DVE
    split the PSUM evacuations so each era stays balanced.
"""

import numpy as np
import ml_dtypes

B, S, H, G, W = 4, 4096, 1024, 256, 8
CHUNK = 1024
NUM_HEADS = 4
HD = H // NUM_HEADS          # 256 per-head dim
HM = H // 2                  # 512 mixer hidden
N_CHUNKS = B * (S // CHUNK)  # 16 independent chunks
N_CORES = 8
CPC = N_CHUNKS // N_CORES    # 2 chunks per core
NT = CHUNK // 128            # 8 token tiles
KH = H // 128                # 8 feature tiles (H)
KM = HM // 128               # 4 feature tiles (HM)
KP = KH // 2                 # 4 double-row K pairs over H
HDP = HD + 1                 # PV output incl denominator column
EPS = 1e-5
SC_RES = 8192.0              # residual-branch scale, absorbed by LN2
SC_Z = 1.0 / 16.0            # extra scale on stored z so fp16 holds (res-m)
SC_MIX8 = 4.0                # fp8 storage scale for mixed
SC_W = 64.0                  # fp8 weight scale
SC_OT = 32.0                 # ocat fp8 range scale (via ones = 1/32)
FP16 = np.float16
FP8 = ml_dtypes.float8_e4m3

_COMPILED = None


def _build():
    import concourse.bass as bass  # noqa: F401
    import concourse.tile as tile
    from concourse import bacc, mybir

    f16 = mybir.dt.float16
    f8 = mybir.dt.float8e4
    f32 = mybir.dt.float32
    Alu = mybir.AluOpType
    Act = mybir.ActivationFunctionType
    DR = mybir.MatmulPerfMode.DoubleRow

    nc = bacc.Bacc("TRN2", target_bir_lowering=False, debug=False,
                   enable_asserts=True, num_devices=N_CORES)

    xt = nc.dram_tensor("xt", [CPC, H, CHUNK], f16, kind="ExternalInput")
    kernT = nc.dram_tensor("kernt", [H, W], f16, kind="ExternalInput")
    w1a = nc.dram_tensor("w1a", [W + 1, HM], f16, kind="ExternalInput")
    gln = nc.dram_tensor("gln", [128, KM], f32, kind="ExternalInput")
    bln = nc.dram_tensor("bln", [128, KM], f32, kind="ExternalInput")
    w2 = nc.dram_tensor("w2", [HM, H], f16, kind="ExternalInput")
    b2c = nc.dram_tensor("b2c", [128, KH], f32, kind="ExternalInput")
    b2r = nc.dram_tensor("b2r", [128, KH], f32, kind="ExternalInput")
    wq8 = nc.dram_tensor("wq8", [H, H], f8, kind="ExternalInput")
    wk8 = nc.dram_tensor("wk8", [H, H], f8, kind="ExternalInput")
    wv8 = nc.dram_tensor("wv8", [H, H], f8, kind="ExternalInput")
    wo8 = nc.dram_tensor("wo8", [H, H], f8, kind="ExternalInput")
    identD = nc.dram_tensor("ident", [128, 128], f16, kind="ExternalInput")
    gw = nc.dram_tensor("gw", [H, G], f16, kind="ExternalInput")
    bw = nc.dram_tensor("bw", [128, G], f16, kind="ExternalInput")
    y = nc.dram_tensor("y", [CPC, CHUNK, G], f32, kind="ExternalOutput")

    with tile.TileContext(nc) as tc:
        with (
            tc.tile_pool(name="wp", bufs=1) as wp,
            tc.tile_pool(name="ws", bufs=1) as ws,
            tc.tile_pool(name="sm", bufs=4) as sm,
            tc.tile_pool(name="ps", bufs=2, space="PSUM") as ps,
            tc.tile_pool(name="ps2", bufs=3, space="PSUM") as ps2,
        ):
            # ---------- persistent weights ----------
            kt_sb = wp.tile([128, KH, W], f16, tag="ktw")
            nc.sync.dma_start(kt_sb[:], kernT.ap().rearrange("(i p) w -> p i w", p=128))
            w1a_sb = wp.tile([W + 1, HM], f16, tag="w1a")
            gln_sb = wp.tile([128, KM], f32, tag="gln")
            bln_sb = wp.tile([128, KM], f32, tag="bln")
            b2_sb = wp.tile([128, KH], f32, tag="b2")
            b2r_sb = wp.tile([128, KH], f32, tag="b2r")
            w2_sb = wp.tile([128, KM, H], f16, tag="w2s")
            wq_sb = wp.tile([128, KH, H], f8, tag="wq")
            wk_sb = wp.tile([128, KH, H], f8, tag="wk")
            wv_sb = wp.tile([128, KH, H], f8, tag="wv")
            wo_sb = wp.tile([128, KH, H], f8, tag="w2s", name="wo_sb")
            id_sb = wp.tile([128, 128], f16, tag="ident")
            bw_sb = wp.tile([128, G], f16, tag="bw")

            def load_w1():
                nc.sync.dma_start(w1a_sb[:], w1a.ap())

            def load_weights():
                # emitted after the input loads so x doesn't queue behind 6MB
                nc.sync.dma_start(gln_sb[:], gln.ap())
                nc.sync.dma_start(bln_sb[:], bln.ap())
                nc.sync.dma_start(b2_sb[:], b2c.ap())
                nc.sync.dma_start(b2r_sb[:], b2r.ap())
                nc.sync.dma_start(w2_sb[:], w2.ap().rearrange("(i p) m -> p i m", p=128))
                nc.sync.dma_start(wq_sb[:], wq8.ap().rearrange("(i p) m -> p i m", p=128))
                nc.sync.dma_start(wk_sb[:], wk8.ap().rearrange("(i p) m -> p i m", p=128))
                nc.sync.dma_start(wv_sb[:], wv8.ap().rearrange("(i p) m -> p i m", p=128))
                nc.sync.dma_start(id_sb[:], identD.ap())
                nc.sync.dma_start(bw_sb[:], bw.ap())
            eps_sb = wp.tile([128, 1], f32, tag="eps")
            nc.vector.memset(eps_sb[:], EPS)
            # LN2 sqrt runs with scale=2^-8 so its output is sqrt(v+eps2)/16;
            # bias carries the same 2^-8.
            eps2_sb = wp.tile([128, 1], f32, tag="eps2")
            nc.vector.memset(eps2_sb[:], EPS * SC_RES * SC_RES / 256.0)

            St = [dict() for _ in range(CPC)]
            Gw = {}

            def load_wo():
                # wo time-shares the w2s slot (w2 dead after st3_mm(1));
                # emitted right after the last w2 GEMM so the SP queue
                # never head-of-line blocks on the WAR wait
                nc.sync.dma_start(wo_sb[:], wo8.ap().rearrange("(i p) m -> p i m", p=128))

            def load_gw():
                # gw time-shares the hidT slot (dead once w2(1) is emitted);
                # padded to the tag's 8KB byte size
                gwp = ws.tile([128, KH, 2, G], f16, tag="hidT", bufs=1,
                              name="gwpad")
                nc.sync.dma_start(gwp[:, :, 0, :],
                                  gw.ap().rearrange("(i p) g -> p i g", p=128))
                Gw["t"] = gwp

            # ---------- stage 1: input load + wavelet coeffs ----------
            def st1_load(c):
                xts = ws.tile([128, KH, CHUNK], f16, tag="A", bufs=2, name="xts")
                for ii in range(KH):
                    nc.sync.dma_start(
                        xts[:, ii:ii + 1, :],
                        xt.ap()[c, ii * 128:(ii + 1) * 128, :].rearrange(
                            "(i p) t -> p i t", p=128))
                St[c]["xts"] = xts

            def st1_mm(c):
                xts = St[c]["xts"]
                coef = ws.tile([W + 1, CHUNK], f16, tag="coef", bufs=1)
                nc.gpsimd.memset(coef[:, :], 1.0)  # row W = folded mix_b1
                for n in range(2):
                    cps = ps.tile([128, 512], f32, tag="mm")
                    for i in range(KH):
                        nc.tensor.matmul(cps[:W, :], kt_sb[:, i, :],
                                         xts[:, i, n * 512:(n + 1) * 512],
                                         start=(i == 0), stop=(i == KH - 1))
                    nc.scalar.copy(coef[:W, n * 512:(n + 1) * 512], cps[:W, :])
                St[c]["coef"] = coef

            # ---------- stage 2a: w1 + stats; hid kept fp16 in SBUF ----------
            def st2a(c, heng):
                coef = St[c]["coef"]
                hid16 = ws.tile([128, NT, HM], f16, tag="O8y", bufs=2)
                mvs = ws.tile([128, NT, 2], f32, tag="MVS", bufs=2)
                for t in range(NT):
                    hps = ps.tile([128, HM], f32, tag="mm")
                    nc.tensor.matmul(hps[:], coef[:, t * 128:(t + 1) * 128],
                                     w1a_sb[:], start=True, stop=True)
                    st6 = sm.tile([128, 6], f32, tag="st6", bufs=2)
                    nc.vector.bn_stats(st6[:], hps[:])
                    nc.vector.bn_aggr(mvs[:, t, :], st6[:])
                    if heng == "a":
                        nc.scalar.copy(hid16[:, t, :], hps[:])
                    else:
                        nc.vector.tensor_copy(hid16[:, t, :], hps[:])
                St[c]["hid16"], St[c]["mvs"] = hid16, mvs

            # ---------- LN1 iv: ONE batched sqrt + reciprocal ----------
            def sqrt_ln1(c):
                mvs = St[c]["mvs"]
                ivs = ws.tile([128, NT, 1], f32, tag="IVS", bufs=2, name="ivs")
                sq = sm.tile([128, NT], f32, tag="sq8", bufs=2)
                nc.scalar.activation(sq[:], mvs[:, :, 1], Act.Sqrt,
                                     bias=eps_sb[:])
                nc.vector.reciprocal(ivs[:, :, 0], sq[:])
                St[c]["ivs"] = ivs

            # ---------- stage 2b: LN1 apply (Pool) + transpose + gelu ----------
            def st2b(c, aeng="vp"):
                hid16, mvs, ivs = St[c]["hid16"], St[c]["mvs"], St[c]["ivs"]
                hidT = ws.tile([128, KM, CHUNK], f16, tag="hidT", bufs=1)
                St[c]["hidT"] = hidT
                for t in range(NT):
                    tmp = sm.tile([128, HM], f16, tag="mtmp", bufs=2)
                    eng = nc.vector if aeng[t % len(aeng)] == "v" else nc.gpsimd
                    eng.tensor_scalar(tmp[:], hid16[:, t, :],
                                      mvs[:, t, 0:1], ivs[:, t, :],
                                      op0=Alu.subtract, op1=Alu.mult)
                    nc.sync.dma_start_transpose(hidT[:, :, t * 128:(t + 1) * 128],
                                                tmp[:])
                # gamma/beta fused into gelu via per-partition scale/bias;
                # per n-half so w2 can start after the first 4 transposes
                for nh in range(2):
                    for ki in range(KM):
                        sl = hidT[:, ki, nh * 512:(nh + 1) * 512]
                        nc.scalar.activation(sl, sl, Act.Gelu,
                                             bias=bln_sb[:, ki:ki + 1],
                                             scale=gln_sb[:, ki:ki + 1])

            # ---------- stage 3: w2 GEMM (fp16) -> mixT16 + mix8 ----------
            def st3_mm(c, m_lo, m_hi, flip=False):
                hidT = St[c]["hidT"]
                if "mixT16" not in St[c]:
                    St[c]["mixT16"] = ws.tile([128, KH, CHUNK], f16, tag="B",
                                              bufs=2, name="mixT16")
                    St[c]["mix8"] = ws.tile([128, KH, CHUNK], f8,
                                            tag="E8" if c == 0 else "O8y",
                                            bufs=3 if c == 0 else 2, name="mix8")
                mixT16, mix8 = St[c]["mixT16"], St[c]["mix8"]
                for m in range(m_lo, m_hi):
                    wide = ps2.tile([128, CHUNK], f32, tag="wide")
                    for n in range(2):
                        mps = wide[:, n * 512:(n + 1) * 512]
                        for ki in range(KM):
                            nc.tensor.matmul(mps, w2_sb[:, ki, m * 128:(m + 1) * 128],
                                             hidT[:, ki, n * 512:(n + 1) * 512],
                                             start=(ki == 0), stop=(ki == KM - 1))
                    # 512-halves evacuated on BOTH engines in parallel so the
                    # psum bank frees fast and the PE p-state stays high
                    if c == 0:
                        # critical path: mix8 read directly from PSUM by DVE
                        # while Act takes mixT16 — no Pool serialization
                        for n in range(2):
                            half = wide[:, n * 512:(n + 1) * 512]
                            nc.scalar.activation(
                                mixT16[:, m, n * 512:(n + 1) * 512], half,
                                Act.Identity, bias=b2r_sb[:, m:m + 1],
                                scale=SC_RES)
                            nc.vector.tensor_scalar(
                                mix8[:, m, n * 512:(n + 1) * 512], half,
                                b2_sb[:, m:m + 1], SC_MIX8,
                                op0=Alu.add, op1=Alu.mult)
                    else:
                        for n in range(2):
                            dst = mixT16[:, m, n * 512:(n + 1) * 512]
                            half = wide[:, n * 512:(n + 1) * 512]
                            if (n == 0) != flip:
                                nc.scalar.activation(dst, half, Act.Identity,
                                                     bias=b2r_sb[:, m:m + 1],
                                                     scale=SC_RES)
                            else:
                                nc.vector.tensor_scalar(dst, half,
                                                        b2r_sb[:, m:m + 1], SC_RES,
                                                        op0=Alu.add, op1=Alu.mult)
                        nc.gpsimd.tensor_scalar(mix8[:, m, :], mixT16[:, m, :],
                                                SC_MIX8 / SC_RES, None, op0=Alu.mult)

            def st3_post(c):
                mixT16 = St[c]["mixT16"]
                mixN = ws.tile([128, NT, H], f16, tag="mixN", bufs=2)
                for m in range(KH):
                    nc.sync.dma_start_transpose(mixN[:, :, m * 128:(m + 1) * 128],
                                                mixT16[:, m, :])
                St[c]["mixN"] = mixN

            # ---------- stage 4: q/k/v projections (fp8 double-row) ----------
            def st4_qk(c, which, engines, m_lo=0, m_hi=KH):
                mix8 = St[c]["mix8"]
                tag = "Q8" if which == "q" else "K8"
                wsb = wq_sb if which == "q" else wk_sb
                if which + "T8" not in St[c]:
                    St[c][which + "T8"] = ws.tile(
                        [128, KH, CHUNK], f8, tag=tag,
                        bufs=1, name=tag)
                dst = St[c][which + "T8"]
                for m in range(m_lo, m_hi):
                    qps = ps2.tile([128, CHUNK], f32, tag="wide")
                    for n in range(2):
                        for kj in range(KP):
                            nc.tensor.matmul(
                                qps[:, n * 512:(n + 1) * 512],
                                wsb[:, 2 * kj:2 * kj + 2, m * 128:(m + 1) * 128],
                                mix8[:, 2 * kj:2 * kj + 2, n * 512:(n + 1) * 512],
                                start=(kj == 0), stop=(kj == KP - 1),
                                perf_mode=DR)
                    for n in range(2):
                        dsl = dst[:, m, n * 512:(n + 1) * 512]
                        half = qps[:, n * 512:(n + 1) * 512]
                        if engines[(2 * m + n) % len(engines)] == "a":
                            nc.scalar.mul(dsl, half, 1.0 / SC_W)
                        else:
                            nc.vector.tensor_scalar(dsl, half, 1.0 / SC_W,
                                                    None, op0=Alu.mult)

            def st4_v(c, engines):
                mix8 = St[c]["mix8"]
                vN8 = ws.tile([128, NT, NUM_HEADS, HDP], f8, tag="V8", bufs=1)
                # ones columns: per-head denominator rides inside PV (1/SC_OT)
                nc.gpsimd.memset(vN8[:, :, :, HD:HDP], 1.0 / SC_OT)
                for t in range(NT):
                    vps = ps2.tile([128, CHUNK], f32, tag="wide")
                    for n in range(2):
                        for kj in range(KP):
                            nc.tensor.matmul(
                                vps[:, n * 512:(n + 1) * 512],
                                mix8[:, 2 * kj:2 * kj + 2, t * 128:(t + 1) * 128],
                                wv_sb[:, 2 * kj:2 * kj + 2, n * 512:(n + 1) * 512],
                                start=(kj == 0), stop=(kj == KP - 1),
                                perf_mode=DR)
                    for n in range(2):
                        src = vps[:, n * 512:(n + 1) * 512].rearrange(
                            "p (h d) -> p h d", h=2)
                        dstv = vN8[:, t, 2 * n:2 * n + 2, :HD]
                        if engines[(2 * t + n) % len(engines)] == "a":
                            nc.scalar.mul(dstv, src, 1.0 / SC_W)
                        else:
                            nc.vector.tensor_scalar(dstv, src, 1.0 / SC_W,
                                                    None, op0=Alu.mult)
                St[c]["vN8"] = vN8

            # ---------- stage 5: attention ----------
            def sc_exp(c, h):
                qT8, kT8 = St[c]["qT8"], St[c]["kT8"]
                et8 = ws.tile([128, KH, CHUNK], f8, tag="E8", bufs=3, name="et8")
                for kt in range(NT):
                    stp = ps2.tile([128, CHUNK], f32, tag="wide")
                    for qn in range(2):
                        nc.tensor.matmul(
                            stp[:, qn * 512:(qn + 1) * 512],
                            kT8[:, 2 * h:2 * h + 2, kt * 128:(kt + 1) * 128],
                            qT8[:, 2 * h:2 * h + 2, qn * 512:(qn + 1) * 512],
                            start=True, stop=True, perf_mode=DR)
                    # psum holds (4q.4k)=16*qk
                    nc.scalar.activation(et8[:, kt, :], stp[:], Act.Exp,
                                         scale=float(HD ** -0.5 / 16.0))
                St[c]["et8%d" % h] = et8

            def pv(c, h, oeng="v", ceng="p", tail=None):
                et8, vN8 = St[c]["et8%d" % h], St[c]["vN8"]
                if "ocat" not in St[c]:
                    St[c]["ocat"] = ws.tile([128, NT, H], f16, tag="A", bufs=2,
                                            name="ocat")
                ocat = St[c]["ocat"]
                last = h == NUM_HEADS - 1
                if last:
                    St[c]["otc"] = ws.tile([128, KH, CHUNK], f16, tag="A",
                                           bufs=2, name="otc")
                    St[c]["otc8"] = ws.tile([128, KH, CHUNK], f8, tag="O8y",
                                            bufs=2, name="otc8")
                for qt in range(NT):
                    ovp = ps.tile([128, 512], f32, tag="mm")
                    for kj in range(KP):
                        nc.tensor.matmul(ovp[:, :HDP],
                                         et8[:, 2 * kj:2 * kj + 2,
                                             qt * 128:(qt + 1) * 128],
                                         vN8[:, 2 * kj:2 * kj + 2, h, :],
                                         start=(kj == 0), stop=(kj == KP - 1),
                                         perf_mode=DR)
                    rq = sm.tile([128, 1], f32, tag="rq")
                    nc.vector.reciprocal(rq[:], ovp[:, HD:HDP])
                    # ocat = SC_OT*SC_MIX8*o (ones=1/32 baked the 32x)
                    if oeng[qt % len(oeng)] == "a":
                        nc.scalar.activation(ocat[:, qt, h * HD:(h + 1) * HD],
                                             ovp[:, :HD], Act.Copy, scale=rq[:])
                    else:
                        nc.vector.tensor_scalar(ocat[:, qt, h * HD:(h + 1) * HD],
                                                ovp[:, :HD], rq[:], None,
                                                op0=Alu.mult)
                    if last:
                        otc, otc8 = St[c]["otc"], St[c]["otc8"]
                        blk = slice(qt * 128, (qt + 1) * 128)
                        nc.sync.dma_start_transpose(otc[:, :, blk], ocat[:, qt, :])
                        # fp8 cast (SBUF->SBUF): Pool normally; Act in the
                        # tail where it is otherwise idle
                        if ceng == "p":
                            nc.gpsimd.tensor_copy(otc8[:, :, blk], otc[:, :, blk])
                        elif ceng == "a":
                            nc.scalar.copy(otc8[:, :, blk], otc[:, :, blk])
                        else:
                            nc.vector.tensor_copy(otc8[:, :, blk], otc[:, :, blk])
                        if tail is not None:
                            tail(qt)

            # ---------- stage 6: wo + residual(PE) + stats + raw z ----------
            def st6_pre(c):
                # chunk1's zT lives in the mixN slot (mixN(0) dead by then) so
                # its transposes can start while ocat(1) is still being read
                St[c]["zT"] = ws.tile([128, KH, CHUNK], f16,
                                      tag="A" if c == 0 else "mixN", bufs=2,
                                      name="zT")
                St[c]["z"] = ws.tile([128, NT, H], f16, tag="B", bufs=2, name="z")
                St[c]["mv2s"] = ws.tile([128, NT, 2], f32, tag="MVS", bufs=2,
                                        name="mv2s")

            def wo_ln2(c, t_lo, t_hi, zeng="v", do_zt=False):
                otc8, mixN = St[c]["otc8"], St[c]["mixN"]
                z, mv2s, zT = St[c]["z"], St[c]["mv2s"], St[c]["zT"]
                for t in range(t_lo, t_hi):
                    ops2 = ps2.tile([128, CHUNK], f32, tag="wide")
                    for n in range(2):
                        for fi in range(KP):
                            nc.tensor.matmul(
                                ops2[:, n * 512:(n + 1) * 512],
                                otc8[:, 2 * fi:2 * fi + 2, t * 128:(t + 1) * 128],
                                wo_sb[:, 2 * fi:2 * fi + 2, n * 512:(n + 1) * 512],
                                start=(fi == 0), stop=False, perf_mode=DR)
                        # residual: += I.T @ mixN on the PE (both SC_RES-scaled)
                        nc.tensor.matmul(ops2[:, n * 512:(n + 1) * 512], id_sb[:],
                                         mixN[:, t, n * 512:(n + 1) * 512],
                                         start=False, stop=True)
                    st6b = sm.tile([128, 2, 6], f32, tag="st6b", bufs=2)
                    for half in range(2):
                        nc.vector.bn_stats(st6b[:, half, :],
                                           ops2[:, half * 512:(half + 1) * 512])
                    nc.vector.bn_aggr(mv2s[:, t, :], st6b[:])
                    # z stored UNNORMALIZED: (res - m)/16; iv lands at st7 evac
                    if zeng[t % len(zeng)] == "v":
                        nc.vector.tensor_scalar(z[:, t, :], ops2[:],
                                                mv2s[:, t, 0:1], SC_Z,
                                                op0=Alu.subtract, op1=Alu.mult)
                    else:
                        nmi = sm.tile([128, 1], f32, tag="nmi", bufs=2)
                        nc.vector.tensor_scalar(nmi[:], mv2s[:, t, 0:1], -SC_Z,
                                                None, op0=Alu.mult)
                        nc.scalar.activation(z[:, t, :], ops2[:], Act.Identity,
                                             bias=nmi[:], scale=SC_Z)
                    if do_zt:
                        nc.sync.dma_start_transpose(
                            zT[:, :, t * 128:(t + 1) * 128], z[:, t, :])

            # ---------- LN2 iv: ONE batched sqrt (era boundary) + recip ------
            def sqrt_ln2(c):
                mv2s = St[c]["mv2s"]
                iv16 = ws.tile([128, NT], f32, tag="IVS", bufs=2, name="iv16")
                sq = sm.tile([128, NT], f32, tag="sq8", bufs=2)
                # sqrt((v+eps2)/256) = sqrt(v+eps2)/16 ; recip -> 16/sqrt(...)
                nc.scalar.activation(sq[:], mv2s[:, :, 1], Act.Sqrt,
                                     bias=eps2_sb[:], scale=1.0 / 256.0)
                nc.vector.reciprocal(iv16[:], sq[:])
                St[c]["iv16"] = iv16

            def zt7(c, t_lo, t_hi):
                zT, z = St[c]["zT"], St[c]["z"]
                for t in range(t_lo, t_hi):
                    nc.sync.dma_start_transpose(zT[:, :, t * 128:(t + 1) * 128],
                                                z[:, t, :])
                st7(c, t_lo, t_hi)

            # ---------- stage 7: output projection (fp16) ----------
            def st7(c, t_lo, t_hi):
                zT, iv16 = St[c]["zT"], St[c]["iv16"]
                if "ych" not in St[c]:
                    St[c]["ych"] = ws.tile([128, NT, G], f32, tag="O8y", bufs=2,
                                           name="ych")
                ych = St[c]["ych"]
                for t in range(t_lo, t_hi, 2):
                    yps = ps2.tile([128, CHUNK], f32, tag="wide")
                    for tt in (t, t + 1):
                        if tt >= t_hi:
                            continue
                        off = (tt - t) * 512
                        for fi in range(KH):
                            nc.tensor.matmul(yps[:, off:off + G],
                                             zT[:, fi, tt * 128:(tt + 1) * 128],
                                             Gw["t"][:, fi, 0, :],
                                             start=(fi == 0), stop=(fi == KH - 1))
                        # y = yraw * (16/sigma) + bias, fused on DVE
                        nc.vector.scalar_tensor_tensor(
                            ych[:, tt, :], yps[:, off:off + G],
                            iv16[:, tt:tt + 1], bw_sb[:],
                            op0=Alu.mult, op1=Alu.add)

            def yout(c):
                ych = St[c]["ych"]
                for hh in range(4):
                    nc.sync.dma_start(
                        y.ap()[c, hh * 256:(hh + 1) * 256, :].rearrange(
                            "(t p) g -> p t g", p=128),
                        ych[:, hh * 2:(hh + 1) * 2, :])

            MARKS = []

            def mark(label):
                MARKS.append((label, len(list(nc.all_instructions()))))
            _build.MARKS = MARKS

            # ================= emission schedule (software pipeline) =========
            mark("st1_load(0)"); st1_load(0)
            mark("load_w1()"); load_w1()
            mark("st1_load(1)"); st1_load(1)
            mark("load_weights()"); load_weights()
            mark("st1_mm(0)"); st1_mm(0)
            mark("st2a(0)"); st2a(0, heng="v")
            mark("st1_mm(1)"); st1_mm(1)
            mark("st2a(1)"); st2a(1, heng="a")
            mark("sqrt_ln1(0)"); sqrt_ln1(0)
            mark("sqrt_ln1(1)"); sqrt_ln1(1)
            mark("st2b(0)"); st2b(0, aeng="vp")
            mark("st3_mm(0)"); st3_mm(0, 0, KH)
            mark("st2b(1)"); st2b(1, aeng="pv")
            mark("st3_post(0)"); st3_post(0)
            # ---- chunk0 q/k/v fused with the start of chunk0 attention ----
            mark("st4_qk(0,q)"); st4_qk(0, "q", "av")
            mark("st4_qk(0,k01)"); st4_qk(0, "k", "va", 0, 2)
            mark("sc_exp(0,0)"); sc_exp(0, 0)
            mark("st4_qk(0,k27)"); st4_qk(0, "k", "va", 2, KH)
            mark("st4_v(0)"); st4_v(0, "av")
            mark("sc_exp(0,1)"); sc_exp(0, 1)
            mark("st3_mm(1,0,4)"); st3_mm(1, 0, 4, flip=True)
            mark("pv(0,0)"); pv(0, 0, oeng="v")
            mark("sc_exp(0,2)"); sc_exp(0, 2)
            mark("st3_mm(1,4,8)"); st3_mm(1, 4, KH, flip=True)
            mark("pv(0,1)"); pv(0, 1, oeng="v")
            mark("st3_post(1)"); st3_post(1); load_wo(); load_gw()
            mark("st6_pre(0)"); st6_pre(0)
            mark("sc_exp(0,3)"); sc_exp(0, 3)
            # chunk1 q/k head-0 tiles first so chunk1's exp era starts early
            mark("st4_qk(1,k)"); st4_qk(1, "k", "av")
            mark("st4_qk(1,q01)"); st4_qk(1, "q", "av", 0, 2)
            mark("sc_exp(1,0)"); sc_exp(1, 0)
            mark("pv(0,2)"); pv(0, 2, oeng="v")
            mark("st4_qk(1,q27)"); st4_qk(1, "q", "av", 2, KH)

            def tail0(qt):
                if qt >= 2:
                    mark(f"wo_ln2(0,{qt-2})"); wo_ln2(0, qt - 2, qt - 1, zeng="v")
            mark("pv(0,3)"); pv(0, 3, oeng="v", tail=tail0)
            mark("wo_ln2(0,6,8)"); wo_ln2(0, 6, NT, zeng="v")
            mark("sqrt_ln2(0)"); sqrt_ln2(0)
            mark("st4_v(1)"); st4_v(1, "av")
            # ---- chunk1 attention; chunk0 output stages fill the gaps ----
            # scores emitted BEFORE chunk0's st7 so the exp stream never
            # waits behind the zT->st7 latency chain on the PE FIFO
            mark("sc_exp(1,1)"); sc_exp(1, 1)
            mark("pv(1,0)"); pv(1, 0, oeng="v")
            mark("zt7(0,0,4)"); zt7(0, 0, 4)
            mark("sc_exp(1,2)"); sc_exp(1, 2)
            mark("pv(1,1)"); pv(1, 1, oeng="v")
            mark("zt7(0,4,8)"); zt7(0, 4, NT)
            mark("yout(0)"); yout(0)
            mark("st6_pre(1)"); st6_pre(1)
            mark("pv(1,2)"); pv(1, 2, oeng="v")

            def tail1(qt):
                if qt >= 2:
                    mark(f"wo_ln2(1,{qt-2})"); wo_ln2(1, qt - 2, qt - 1,
                                                     zeng="a", do_zt=True)
            mark("sc_exp(1,3)"); sc_exp(1, 3)
            mark("pv(1,3)"); pv(1, 3, oeng="v", ceng="a", tail=tail1)
            mark("wo_ln2(1,6,8)"); wo_ln2(1, 6, NT, zeng="a", do_zt=True)
            mark("sqrt_ln2(1)"); sqrt_ln2(1)
            mark("st7(1,0,4)"); st7(1, 0, 4)
            mark("st7(1,4,8)"); st7(1, 4, NT)
            mark("yout(1)"); yout(1)

    nc.compile()
    return nc


def _get_compiled():
    global _COMPILED
    if _COMPILED is None:
        _COMPILED = _build()
    return _COMPILED


def _prep_inputs(inputs):
    f32 = np.float32

    def a(name):
        return np.asarray(inputs[name], dtype=f32)

    x = a("x")
    mw = a("mother_wavelets")
    scales = a("scales")
    norm = np.sqrt(np.sum(mw ** 2, axis=2, keepdims=True))
    kern = (mw / np.maximum(norm, 1e-12)) * (1.0 / (1.0 + np.exp(-scales)))
    kern = kern[0, :, :, 0]                      # (W, H)
    kernT = np.ascontiguousarray(kern.T).astype(FP16)

    w1a = np.concatenate([a("mix_w1"), a("mix_b1")[None, :]], axis=0).astype(FP16)
    gln = np.ascontiguousarray(a("mix_ln_g").reshape(KM, 128).T).astype(f32)
    bln = np.ascontiguousarray(a("mix_ln_b").reshape(KM, 128).T).astype(f32)
    w2 = a("mix_w2").astype(FP16)
    b2c = np.ascontiguousarray(a("mix_b2").reshape(KH, 128).T).astype(f32)
    b2r = (b2c * SC_RES).astype(f32)
    gw = (a("out_ln_g")[:, None] * a("out_w")).astype(FP16)
    bw_vec = a("out_ln_b") @ a("out_w") + a("out_b")
    bw = np.tile(bw_vec[None, :], (128, 1)).astype(FP16)

    shared = {
        "kernt": kernT, "w1a": w1a, "gln": gln, "bln": bln, "w2": w2,
        "b2c": b2c, "b2r": b2r,
        "wq8": (a("wq") * SC_W).astype(FP8), "wk8": (a("wk") * SC_W).astype(FP8),
        "wv8": (a("wv") * SC_W).astype(FP8), "wo8": (a("wo") * SC_W).astype(FP8),
        "ident": np.eye(128, dtype=FP16),
        "gw": gw, "bw": bw,
    }

    xc = x.reshape(N_CHUNKS, CHUNK, H)
    xt_all = np.ascontiguousarray(xc.transpose(0, 2, 1)).astype(FP16)  # (16, H, CHUNK)
    in_maps = []
    for core in range(N_CORES):
        m = dict(shared)
        m["xt"] = np.ascontiguousarray(xt_all[core * CPC:(core + 1) * CPC])
        in_maps.append(m)
    return in_maps


def kernel(**inputs) -> np.ndarray:
    from concourse.bass_utils import run_bass_kernel_spmd

    nc = _get_compiled()
    in_maps = _prep_inputs(inputs)
    res = run_bass_kernel_spmd(nc, in_maps, core_ids=list(range(N_CORES)))
    out = np.concatenate([r["y"] for r in res.results], axis=0)  # (16, CHUNK, G)
    return out.reshape(B, S, G).astype(np.float32)


# revision 3
# speedup vs baseline: 1.0151x; 1.0151x over previous
"""Trainium2 Bass kernel for nn_EntropyLM — v2 (see kernel.py for v1).

Changes vs v1 (262.5us):
  * V-ones: per-head ones column rides inside vN8 (layout [128, NT, 4, HD+1])
    so the softmax denominator accumulates in the same PSUM group as PV —
    kills 256 denominator matmuls + ldweights per core.
  * LN2 decoupled from the critical path: z is stored UNNORMALIZED
    ((res - mean)/16 in fp16); 1/sqrt(var) is applied per-token at the st7
    PSUM evacuation (tokens are partitions there) fused with the bias via
    scalar_tensor_tensor.  The sqrt is ONE batched [128,8] Act op per chunk
    at an exp-era boundary; same batching for LN1's iv.
  * Act table eras: sqrt(LN1 c0,c1) -> gelu(c0,c1) -> exp(c0) -> sqrt(LN2 c0)
    -> exp(c1) -> sqrt(LN2 c1): 6 table loads (was 13).
  * Wide 1024-col PSUM evacuations for q/k/v and w2; Pool (no PSUM port)
    takes the SBUF->SBUF work (LN1 apply, mix8 cast, otc8 cast); Act and DVE
    split the PSUM evacuations so each era stays balanced.
"""

import numpy as np
import ml_dtypes

B, S, H, G, W = 4, 4096, 1024, 256, 8
CHUNK = 1024
NUM_HEADS = 4
HD = H // NUM_HEADS          # 256 per-head dim
HM = H // 2                  # 512 mixer hidden
N_CHUNKS = B * (S // CHUNK)  # 16 independent chunks
N_CORES = 8
CPC = N_CHUNKS // N_CORES    # 2 chunks per core
NT = CHUNK // 128            # 8 token tiles
KH = H // 128                # 8 feature tiles (H)
KM = HM // 128               # 4 feature tiles (HM)
KP = KH // 2                 # 4 double-row K pairs over H
HDP = HD + 1                 # PV output incl denominator column
EPS = 1e-5
SC_RES = 8192.0              # residual-branch scale, absorbed by LN2
SC_Z = 1.0 / 16.0            # extra scale on stored z so fp16 holds (res-m)
SC_MIX8 = 4.0                # fp8 storage scale for mixed
SC_W = 64.0                  # fp8 weight scale
SC_OT = 32.0                 # ocat fp8 range scale (via ones = 1/32)
FP16 = np.float16
FP8 = ml_dtypes.float8_e4m3

_COMPILED = None


def _build():
    import concourse.bass as bass  # noqa: F401
    import concourse.tile as tile
    from concourse import bacc, mybir

    f16 = mybir.dt.float16
    f8 = mybir.dt.float8e4
    f32 = mybir.dt.float32
    Alu = mybir.AluOpType
    Act = mybir.ActivationFunctionType
    DR = mybir.MatmulPerfMode.DoubleRow

    nc = bacc.Bacc("TRN2", target_bir_lowering=False, debug=False,
                   enable_asserts=True, num_devices=N_CORES)

    xt = nc.dram_tensor("xt", [CPC, H, CHUNK], f16, kind="ExternalInput")
    kernT = nc.dram_tensor("kernt", [H, W], f16, kind="ExternalInput")
    w1a = nc.dram_tensor("w1a", [W + 1, HM], f16, kind="ExternalInput")
    gln = nc.dram_tensor("gln", [128, KM], f32, kind="ExternalInput")
    bln = nc.dram_tensor("bln", [128, KM], f32, kind="ExternalInput")
    w2 = nc.dram_tensor("w2", [HM, H], f16, kind="ExternalInput")
    b2c = nc.dram_tensor("b2c", [128, KH], f32, kind="ExternalInput")
    b2r = nc.dram_tensor("b2r", [128, KH], f32, kind="ExternalInput")
    wq8 = nc.dram_tensor("wq8", [H, H], f8, kind="ExternalInput")
    wk8 = nc.dram_tensor("wk8", [H, H], f8, kind="ExternalInput")
    wv8 = nc.dram_tensor("wv8", [H, H], f8, kind="ExternalInput")
    wo8 = nc.dram_tensor("wo8", [H, H], f8, kind="ExternalInput")
    identD = nc.dram_tensor("ident", [128, 128], f16, kind="ExternalInput")
    gw = nc.dram_tensor("gw", [H, G], f16, kind="ExternalInput")
    bw = nc.dram_tensor("bw", [128, G], f16, kind="ExternalInput")
    y = nc.dram_tensor("y", [CPC, CHUNK, G], f32, kind="ExternalOutput")

    with tile.TileContext(nc) as tc:
        with (
            tc.tile_pool(name="wp", bufs=1) as wp,
            tc.tile_pool(name="ws", bufs=1) as ws,
            tc.tile_pool(name="sm", bufs=4) as sm,
            tc.tile_pool(name="ps", bufs=2, space="PSUM") as ps,
            tc.tile_pool(name="ps2", bufs=3, space="PSUM") as ps2,
        ):
            # ---------- persistent weights ----------
            kt_sb = wp.tile([128, KH, W], f16, tag="ktw")
            nc.sync.dma_start(kt_sb[:], kernT.ap().rearrange("(i p) w -> p i w", p=128))
            w1a_sb = wp.tile([W + 1, HM], f16, tag="w1a")
            gln_sb = wp.tile([128, KM], f32, tag="gln")
            bln_sb = wp.tile([128, KM], f32, tag="bln")
            b2_sb = wp.tile([128, KH], f32, tag="b2")
            b2r_sb = wp.tile([128, KH], f32, tag="b2r")
            w2_sb = wp.tile([128, KM, H], f16, tag="w2s")
            wq_sb = wp.tile([128, KH, H], f8, tag="wq")
            wk_sb = wp.tile([128, KH, H], f8, tag="wk")
            wv_sb = wp.tile([128, KH, H], f8, tag="wv")
            wo_sb = wp.tile([128, KH, H], f8, tag="w2s", name="wo_sb")
            id_sb = wp.tile([128, 128], f16, tag="ident")
            bw_sb = wp.tile([128, G], f16, tag="bw")

            def load_w1():
                nc.sync.dma_start(w1a_sb[:], w1a.ap())

            def load_weights():
                # emitted after the input loads so x doesn't queue behind 6MB
                nc.sync.dma_start(gln_sb[:], gln.ap())
                nc.sync.dma_start(bln_sb[:], bln.ap())
                nc.sync.dma_start(b2_sb[:], b2c.ap())
                nc.sync.dma_start(b2r_sb[:], b2r.ap())
                nc.sync.dma_start(w2_sb[:], w2.ap().rearrange("(i p) m -> p i m", p=128))
                nc.sync.dma_start(wq_sb[:], wq8.ap().rearrange("(i p) m -> p i m", p=128))
                nc.sync.dma_start(wk_sb[:], wk8.ap().rearrange("(i p) m -> p i m", p=128))
                nc.sync.dma_start(wv_sb[:], wv8.ap().rearrange("(i p) m -> p i m", p=128))
                nc.sync.dma_start(id_sb[:], identD.ap())
                nc.sync.dma_start(bw_sb[:], bw.ap())
            eps_sb = wp.tile([128, 1], f32, tag="eps")
            nc.vector.memset(eps_sb[:], EPS)
            # LN2 sqrt runs with scale=2^-8 so its output is sqrt(v+eps2)/16;
            # bias carries the same 2^-8.
            eps2_sb = wp.tile([128, 1], f32, tag="eps2")
            nc.vector.memset(eps2_sb[:], EPS * SC_RES * SC_RES / 256.0)

            St = [dict() for _ in range(CPC)]
            Gw = {}

            def load_wo():
                # wo time-shares the w2s slot (w2 dead after st3_mm(1));
                # emitted right after the last w2 GEMM so the SP queue
                # never head-of-line blocks on the WAR wait
                nc.sync.dma_start(wo_sb[:], wo8.ap().rearrange("(i p) m -> p i m", p=128))

            def load_gw():
                # gw time-shares the hidT slot (dead once w2(1) is emitted);
                # padded to the tag's 8KB byte size
                gwp = ws.tile([128, KH, 2, G], f16, tag="hidT", bufs=1,
                              name="gwpad")
                nc.sync.dma_start(gwp[:, :, 0, :],
                                  gw.ap().rearrange("(i p) g -> p i g", p=128))
                Gw["t"] = gwp

            # ---------- stage 1: input load + wavelet coeffs ----------
            def st1_load(c):
                xts = ws.tile([128, KH, CHUNK], f16, tag="A", bufs=2, name="xts")
                for ii in range(KH):
                    nc.sync.dma_start(
                        xts[:, ii:ii + 1, :],
                        xt.ap()[c, ii * 128:(ii + 1) * 128, :].rearrange(
                            "(i p) t -> p i t", p=128))
                St[c]["xts"] = xts

            def st1_mm(c):
                xts = St[c]["xts"]
                coef = ws.tile([W + 1, CHUNK], f16, tag="coef", bufs=1)
                nc.gpsimd.memset(coef[:, :], 1.0)  # row W = folded mix_b1
                for n in range(2):
                    cps = ps.tile([128, 512], f32, tag="mm")
                    for i in range(KH):
                        nc.tensor.matmul(cps[:W, :], kt_sb[:, i, :],
                                         xts[:, i, n * 512:(n + 1) * 512],
                                         start=(i == 0), stop=(i == KH - 1))
                    nc.scalar.copy(coef[:W, n * 512:(n + 1) * 512], cps[:W, :])
                St[c]["coef"] = coef

            # ---------- stage 2a: w1 + stats; hid kept fp16 in SBUF ----------
            def st2a(c, heng):
                coef = St[c]["coef"]
                hid16 = ws.tile([128, NT, HM], f16, tag="O8y", bufs=2)
                mvs = ws.tile([128, NT, 2], f32, tag="MVS", bufs=2)
                for t in range(NT):
                    hps = ps.tile([128, HM], f32, tag="mm")
                    nc.tensor.matmul(hps[:], coef[:, t * 128:(t + 1) * 128],
                                     w1a_sb[:], start=True, stop=True)
                    st6 = sm.tile([128, 6], f32, tag="st6", bufs=2)
                    nc.vector.bn_stats(st6[:], hps[:])
                    nc.vector.bn_aggr(mvs[:, t, :], st6[:])
                    if heng == "a":
                        nc.scalar.copy(hid16[:, t, :], hps[:])
                    else:
                        nc.vector.tensor_copy(hid16[:, t, :], hps[:])
                St[c]["hid16"], St[c]["mvs"] = hid16, mvs

            # ---------- LN1 iv: ONE batched sqrt + reciprocal ----------
            def sqrt_ln1(c):
                mvs = St[c]["mvs"]
                ivs = ws.tile([128, NT, 1], f32, tag="IVS", bufs=2, name="ivs")
                sq = sm.tile([128, NT], f32, tag="sq8", bufs=2)
                nc.scalar.activation(sq[:], mvs[:, :, 1], Act.Sqrt,
                                     bias=eps_sb[:])
                nc.vector.reciprocal(ivs[:, :, 0], sq[:])
                St[c]["ivs"] = ivs

            # ---------- stage 2b: LN1 apply (Pool) + transpose + gelu ----------
            def st2b(c, aeng="vp"):
                hid16, mvs, ivs = St[c]["hid16"], St[c]["mvs"], St[c]["ivs"]
                hidT = ws.tile([128, KM, CHUNK], f16, tag="hidT", bufs=1)
                St[c]["hidT"] = hidT
                for t in range(NT):
                    tmp = sm.tile([128, HM], f16, tag="mtmp", bufs=3)
                    eng = nc.vector if aeng[t % len(aeng)] == "v" else nc.gpsimd
                    eng.tensor_scalar(tmp[:], hid16[:, t, :],
                                      mvs[:, t, 0:1], ivs[:, t, :],
                                      op0=Alu.subtract, op1=Alu.mult)
                    nc.sync.dma_start_transpose(hidT[:, :, t * 128:(t + 1) * 128],
                                                tmp[:])
                # gamma/beta fused into gelu via per-partition scale/bias;
                # per n-half so w2 can start after the first 4 transposes
                for nh in range(2):
                    for ki in range(KM):
                        sl = hidT[:, ki, nh * 512:(nh + 1) * 512]
                        nc.scalar.activation(sl, sl, Act.Gelu,
                                             bias=bln_sb[:, ki:ki + 1],
                                             scale=gln_sb[:, ki:ki + 1])

            # ---------- stage 3: w2 GEMM (fp16) -> mixT16 + mix8 ----------
            def st3_mm(c, m_lo, m_hi, flip=False):
                hidT = St[c]["hidT"]
                if "mixT16" not in St[c]:
                    St[c]["mixT16"] = ws.tile([128, KH, CHUNK], f16, tag="B",
                                              bufs=2, name="mixT16")
                    St[c]["mix8"] = ws.tile([128, KH, CHUNK], f8,
                                            tag="E8" if c == 0 else "O8y",
                                            bufs=3 if c == 0 else 2, name="mix8")
                mixT16, mix8 = St[c]["mixT16"], St[c]["mix8"]
                for m in range(m_lo, m_hi):
                    wide = ps2.tile([128, CHUNK], f32, tag="wide")
                    for n in range(2):
                        mps = wide[:, n * 512:(n + 1) * 512]
                        for ki in range(KM):
                            nc.tensor.matmul(mps, w2_sb[:, ki, m * 128:(m + 1) * 128],
                                             hidT[:, ki, n * 512:(n + 1) * 512],
                                             start=(ki == 0), stop=(ki == KM - 1))
                    # 512-halves evacuated on BOTH engines in parallel so the
                    # psum bank frees fast and the PE p-state stays high
                    if c == 0:
                        # critical path: mix8 read directly from PSUM by DVE
                        # while Act takes mixT16 — no Pool serialization
                        for n in range(2):
                            half = wide[:, n * 512:(n + 1) * 512]
                            nc.scalar.activation(
                                mixT16[:, m, n * 512:(n + 1) * 512], half,
                                Act.Identity, bias=b2r_sb[:, m:m + 1],
                                scale=SC_RES)
                            nc.vector.tensor_scalar(
                                mix8[:, m, n * 512:(n + 1) * 512], half,
                                b2_sb[:, m:m + 1], SC_MIX8,
                                op0=Alu.add, op1=Alu.mult)
                    else:
                        for n in range(2):
                            dst = mixT16[:, m, n * 512:(n + 1) * 512]
                            half = wide[:, n * 512:(n + 1) * 512]
                            if (n == 0) != flip:
                                nc.scalar.activation(dst, half, Act.Identity,
                                                     bias=b2r_sb[:, m:m + 1],
                                                     scale=SC_RES)
                            else:
                                nc.vector.tensor_scalar(dst, half,
                                                        b2r_sb[:, m:m + 1], SC_RES,
                                                        op0=Alu.add, op1=Alu.mult)
                        nc.gpsimd.tensor_scalar(mix8[:, m, :], mixT16[:, m, :],
                                                SC_MIX8 / SC_RES, None, op0=Alu.mult)

            def st3_post(c):
                mixT16 = St[c]["mixT16"]
                mixN = ws.tile([128, NT, H], f16, tag="mixN", bufs=2)
                for m in range(KH):
                    nc.sync.dma_start_transpose(mixN[:, :, m * 128:(m + 1) * 128],
                                                mixT16[:, m, :])
                St[c]["mixN"] = mixN

            # ---------- stage 4: q/k/v projections (fp8 double-row) ----------
            def st4_qk(c, which, engines, m_lo=0, m_hi=KH):
                mix8 = St[c]["mix8"]
                tag = "Q8" if which == "q" else "K8"
                wsb = wq_sb if which == "q" else wk_sb
                if which + "T8" not in St[c]:
                    St[c][which + "T8"] = ws.tile(
                        [128, KH, CHUNK], f8, tag=tag,
                        bufs=1, name=tag)
                dst = St[c][which + "T8"]
                for m in range(m_lo, m_hi):
                    qps = ps2.tile([128, CHUNK], f32, tag="wide")
                    for n in range(2):
                        for kj in range(KP):
                            nc.tensor.matmul(
                                qps[:, n * 512:(n + 1) * 512],
                                wsb[:, 2 * kj:2 * kj + 2, m * 128:(m + 1) * 128],
                                mix8[:, 2 * kj:2 * kj + 2, n * 512:(n + 1) * 512],
                                start=(kj == 0), stop=(kj == KP - 1),
                                perf_mode=DR)
                    for n in range(2):
                        dsl = dst[:, m, n * 512:(n + 1) * 512]
                        half = qps[:, n * 512:(n + 1) * 512]
                        if engines[(2 * m + n) % len(engines)] == "a":
                            nc.scalar.mul(dsl, half, 1.0 / SC_W)
                        else:
                            nc.vector.tensor_scalar(dsl, half, 1.0 / SC_W,
                                                    None, op0=Alu.mult)

            def st4_v(c, engines):
                mix8 = St[c]["mix8"]
                vN8 = ws.tile([128, NT, NUM_HEADS, HDP], f8, tag="V8", bufs=1)
                # ones columns: per-head denominator rides inside PV (1/SC_OT)
                nc.gpsimd.memset(vN8[:, :, :, HD:HDP], 1.0 / SC_OT)
                for t in range(NT):
                    vps = ps2.tile([128, CHUNK], f32, tag="wide")
                    for n in range(2):
                        for kj in range(KP):
                            nc.tensor.matmul(
                                vps[:, n * 512:(n + 1) * 512],
                                mix8[:, 2 * kj:2 * kj + 2, t * 128:(t + 1) * 128],
                                wv_sb[:, 2 * kj:2 * kj + 2, n * 512:(n + 1) * 512],
                                start=(kj == 0), stop=(kj == KP - 1),
                                perf_mode=DR)
                    for n in range(2):
                        src = vps[:, n * 512:(n + 1) * 512].rearrange(
                            "p (h d) -> p h d", h=2)
                        dstv = vN8[:, t, 2 * n:2 * n + 2, :HD]
                        if engines[(2 * t + n) % len(engines)] == "a":
                            nc.scalar.mul(dstv, src, 1.0 / SC_W)
                        else:
                            nc.vector.tensor_scalar(dstv, src, 1.0 / SC_W,
                                                    None, op0=Alu.mult)
                St[c]["vN8"] = vN8

            # ---------- stage 5: attention ----------
            def sc_exp(c, h):
                qT8, kT8 = St[c]["qT8"], St[c]["kT8"]
                et8 = ws.tile([128, KH, CHUNK], f8, tag="E8", bufs=3, name="et8")
                for kt in range(NT):
                    stp = ps2.tile([128, CHUNK], f32, tag="wide")
                    for qn in range(2):
                        nc.tensor.matmul(
                            stp[:, qn * 512:(qn + 1) * 512],
                            kT8[:, 2 * h:2 * h + 2, kt * 128:(kt + 1) * 128],
                            qT8[:, 2 * h:2 * h + 2, qn * 512:(qn + 1) * 512],
                            start=True, stop=True, perf_mode=DR)
                    # psum holds (4q.4k)=16*qk
                    nc.scalar.activation(et8[:, kt, :], stp[:], Act.Exp,
                                         scale=float(HD ** -0.5 / 16.0))
                St[c]["et8%d" % h] = et8

            def pv(c, h, oeng="v", ceng="p", tail=None):
                et8, vN8 = St[c]["et8%d" % h], St[c]["vN8"]
                if "ocat" not in St[c]:
                    St[c]["ocat"] = ws.tile([128, NT, H], f16, tag="A", bufs=2,
                                            name="ocat")
                ocat = St[c]["ocat"]
                last = h == NUM_HEADS - 1
                if last:
                    St[c]["otc"] = ws.tile([128, KH, CHUNK], f16, tag="A",
                                           bufs=2, name="otc")
                    St[c]["otc8"] = ws.tile([128, KH, CHUNK], f8, tag="O8y",
                                            bufs=2, name="otc8")
                for qt in range(NT):
                    ovp = ps.tile([128, 512], f32, tag="mm")
                    for kj in range(KP):
                        nc.tensor.matmul(ovp[:, :HDP],
                                         et8[:, 2 * kj:2 * kj + 2,
                                             qt * 128:(qt + 1) * 128],
                                         vN8[:, 2 * kj:2 * kj + 2, h, :],
                                         start=(kj == 0), stop=(kj == KP - 1),
                                         perf_mode=DR)
                    rq = sm.tile([128, 1], f32, tag="rq")
                    nc.vector.reciprocal(rq[:], ovp[:, HD:HDP])
                    # ocat = SC_OT*SC_MIX8*o (ones=1/32 baked the 32x)
                    if oeng[qt % len(oeng)] == "a":
                        nc.scalar.activation(ocat[:, qt, h * HD:(h + 1) * HD],
                                             ovp[:, :HD], Act.Copy, scale=rq[:])
                    else:
                        nc.vector.tensor_scalar(ocat[:, qt, h * HD:(h + 1) * HD],
                                                ovp[:, :HD], rq[:], None,
                                                op0=Alu.mult)
                    if last:
                        otc, otc8 = St[c]["otc"], St[c]["otc8"]
                        blk = slice(qt * 128, (qt + 1) * 128)
                        nc.sync.dma_start_transpose(otc[:, :, blk], ocat[:, qt, :])
                        # fp8 cast (SBUF->SBUF): Pool normally; Act in the
                        # tail where it is otherwise idle
                        if ceng == "p":
                            nc.gpsimd.tensor_copy(otc8[:, :, blk], otc[:, :, blk])
                        elif ceng == "a":
                            nc.scalar.copy(otc8[:, :, blk], otc[:, :, blk])
                        else:
                            nc.vector.tensor_copy(otc8[:, :, blk], otc[:, :, blk])
                        if tail is not None:
                            tail(qt)

            # ---------- stage 6: wo + residual(PE) + stats + raw z ----------
            def st6_pre(c):
                # chunk1's zT lives in the mixN slot (mixN(0) dead by then) so
                # its transposes can start while ocat(1) is still being read
                St[c]["zT"] = ws.tile([128, KH, CHUNK], f16,
                                      tag="A" if c == 0 else "mixN", bufs=2,
                                      name="zT")
                St[c]["z"] = ws.tile([128, NT, H], f16, tag="B", bufs=2, name="z")
                St[c]["mv2s"] = ws.tile([128, NT, 2], f32, tag="MVS", bufs=2,
                                        name="mv2s")

            def wo_ln2(c, t_lo, t_hi, zeng="v", do_zt=False):
                otc8, mixN = St[c]["otc8"], St[c]["mixN"]
                z, mv2s, zT = St[c]["z"], St[c]["mv2s"], St[c]["zT"]
                for t in range(t_lo, t_hi):
                    ops2 = ps2.tile([128, CHUNK], f32, tag="wide")
                    for n in range(2):
                        for fi in range(KP):
                            nc.tensor.matmul(
                                ops2[:, n * 512:(n + 1) * 512],
                                otc8[:, 2 * fi:2 * fi + 2, t * 128:(t + 1) * 128],
                                wo_sb[:, 2 * fi:2 * fi + 2, n * 512:(n + 1) * 512],
                                start=(fi == 0), stop=False, perf_mode=DR)
                        # residual: += I.T @ mixN on the PE (both SC_RES-scaled)
                        nc.tensor.matmul(ops2[:, n * 512:(n + 1) * 512], id_sb[:],
                                         mixN[:, t, n * 512:(n + 1) * 512],
                                         start=False, stop=True)
                    st6b = sm.tile([128, 2, 6], f32, tag="st6b", bufs=2)
                    for half in range(2):
                        nc.vector.bn_stats(st6b[:, half, :],
                                           ops2[:, half * 512:(half + 1) * 512])
                    nc.vector.bn_aggr(mv2s[:, t, :], st6b[:])
                    # z stored UNNORMALIZED: (res - m)/16; iv lands at st7 evac
                    if zeng[t % len(zeng)] == "v":
                        nc.vector.tensor_scalar(z[:, t, :], ops2[:],
                                                mv2s[:, t, 0:1], SC_Z,
                                                op0=Alu.subtract, op1=Alu.mult)
                    else:
                        nmi = sm.tile([128, 1], f32, tag="nmi", bufs=2)
                        nc.vector.tensor_scalar(nmi[:], mv2s[:, t, 0:1], -SC_Z,
                                                None, op0=Alu.mult)
                        nc.scalar.activation(z[:, t, :], ops2[:], Act.Identity,
                                             bias=nmi[:], scale=SC_Z)
                    if do_zt:
                        nc.sync.dma_start_transpose(
                            zT[:, :, t * 128:(t + 1) * 128], z[:, t, :])

            # ---------- LN2 iv: ONE batched sqrt (era boundary) + recip ------
            def sqrt_ln2(c):
                mv2s = St[c]["mv2s"]
                iv16 = ws.tile([128, NT], f32, tag="IVS", bufs=2, name="iv16")
                sq = sm.tile([128, NT], f32, tag="sq8", bufs=2)
                # sqrt((v+eps2)/256) = sqrt(v+eps2)/16 ; recip -> 16/sqrt(...)
                nc.scalar.activation(sq[:], mv2s[:, :, 1], Act.Sqrt,
                                     bias=eps2_sb[:], scale=1.0 / 256.0)
                nc.vector.reciprocal(iv16[:], sq[:])
                St[c]["iv16"] = iv16

            def zt7(c, t_lo, t_hi):
                zT, z = St[c]["zT"], St[c]["z"]
                for t in range(t_lo, t_hi):
                    nc.sync.dma_start_transpose(zT[:, :, t * 128:(t + 1) * 128],
                                                z[:, t, :])
                st7(c, t_lo, t_hi)

            # ---------- stage 7: output projection (fp16) ----------
            def st7(c, t_lo, t_hi):
                zT, iv16 = St[c]["zT"], St[c]["iv16"]
                if "ych" not in St[c]:
                    St[c]["ych"] = ws.tile([128, NT, G], f32, tag="O8y", bufs=2,
                                           name="ych")
                ych = St[c]["ych"]
                for t in range(t_lo, t_hi, 2):
                    yps = ps2.tile([128, CHUNK], f32, tag="wide")
                    for tt in (t, t + 1):
                        if tt >= t_hi:
                            continue
                        off = (tt - t) * 512
                        for fi in range(KH):
                            nc.tensor.matmul(yps[:, off:off + G],
                                             zT[:, fi, tt * 128:(tt + 1) * 128],
                                             Gw["t"][:, fi, 0, :],
                                             start=(fi == 0), stop=(fi == KH - 1))
                        # y = yraw * (16/sigma) + bias, fused on DVE
                        nc.vector.scalar_tensor_tensor(
                            ych[:, tt, :], yps[:, off:off + G],
                            iv16[:, tt:tt + 1], bw_sb[:],
                            op0=Alu.mult, op1=Alu.add)

            def yout(c):
                ych = St[c]["ych"]
                for hh in range(4):
                    nc.sync.dma_start(
                        y.ap()[c, hh * 256:(hh + 1) * 256, :].rearrange(
                            "(t p) g -> p t g", p=128),
                        ych[:, hh * 2:(hh + 1) * 2, :])

            MARKS = []

            def mark(label):
                MARKS.append((label, len(list(nc.all_instructions()))))
            _build.MARKS = MARKS

            # ================= emission schedule (software pipeline) =========
            mark("st1_load(0)"); st1_load(0)
            mark("load_w1()"); load_w1()
            mark("st1_load(1)"); st1_load(1)
            mark("load_weights()"); load_weights()
            mark("st1_mm(0)"); st1_mm(0)
            mark("st2a(0)"); st2a(0, heng="v")
            mark("st1_mm(1)"); st1_mm(1)
            mark("st2a(1)"); st2a(1, heng="a")
            mark("sqrt_ln1(0)"); sqrt_ln1(0)
            mark("sqrt_ln1(1)"); sqrt_ln1(1)
            mark("st2b(0)"); st2b(0, aeng="vp")
            mark("st3_mm(0)"); st3_mm(0, 0, KH)
            mark("st2b(1)"); st2b(1, aeng="pv")
            mark("st3_post(0)"); st3_post(0)
            # ---- chunk0 q/k/v fused with the start of chunk0 attention ----
            mark("st4_qk(0,q)"); st4_qk(0, "q", "av")
            mark("st4_qk(0,k01)"); st4_qk(0, "k", "va", 0, 2)
            mark("sc_exp(0,0)"); sc_exp(0, 0)
            mark("st4_qk(0,k27)"); st4_qk(0, "k", "va", 2, KH)
            mark("st4_v(0)"); st4_v(0, "av")
            mark("sc_exp(0,1)"); sc_exp(0, 1)
            mark("st3_mm(1,0,4)"); st3_mm(1, 0, 4, flip=True)
            mark("pv(0,0)"); pv(0, 0, oeng="v")
            mark("sc_exp(0,2)"); sc_exp(0, 2)
            mark("st3_mm(1,4,8)"); st3_mm(1, 4, KH, flip=True)
            mark("pv(0,1)"); pv(0, 1, oeng="v")
            mark("st3_post(1)"); st3_post(1); load_wo(); load_gw()
            mark("st6_pre(0)"); st6_pre(0)
            mark("sc_exp(0,3)"); sc_exp(0, 3)
            # chunk1 q/k head-0 tiles first so chunk1's exp era starts early
            mark("st4_qk(1,k)"); st4_qk(1, "k", "av")
            mark("st4_qk(1,q01)"); st4_qk(1, "q", "av", 0, 2)
            mark("sc_exp(1,0)"); sc_exp(1, 0)
            mark("pv(0,2)"); pv(0, 2, oeng="v")
            mark("st4_qk(1,q27)"); st4_qk(1, "q", "av", 2, KH)

            def tail0(qt):
                if qt >= 2:
                    mark(f"wo_ln2(0,{qt-2})"); wo_ln2(0, qt - 2, qt - 1, zeng="v")
            mark("pv(0,3)"); pv(0, 3, oeng="v", tail=tail0)
            mark("wo_ln2(0,6,8)"); wo_ln2(0, 6, NT, zeng="v")
            mark("sqrt_ln2(0)"); sqrt_ln2(0)
            mark("st4_v(1)"); st4_v(1, "av")
            # ---- chunk1 attention; chunk0 output stages fill the gaps ----
            # scores emitted BEFORE chunk0's st7 so the exp stream never
            # waits behind the zT->st7 latency chain on the PE FIFO
            mark("sc_exp(1,1)"); sc_exp(1, 1)
            mark("pv(1,0)"); pv(1, 0, oeng="v")
            mark("zt7(0,0,4)"); zt7(0, 0, 4)
            mark("sc_exp(1,2)"); sc_exp(1, 2)
            mark("pv(1,1)"); pv(1, 1, oeng="v")
            mark("zt7(0,4,8)"); zt7(0, 4, NT)
            mark("yout(0)"); yout(0)
            mark("st6_pre(1)"); st6_pre(1)
            mark("pv(1,2)"); pv(1, 2, oeng="v")

            def tail1(qt):
                if qt >= 2:
                    mark(f"wo_ln2(1,{qt-2})"); wo_ln2(1, qt - 2, qt - 1,
                                                     zeng="a", do_zt=True)
            mark("sc_exp(1,3)"); sc_exp(1, 3)
            mark("pv(1,3)"); pv(1, 3, oeng="v", ceng="a", tail=tail1)
            mark("wo_ln2(1,6,8)"); wo_ln2(1, 6, NT, zeng="a", do_zt=True)
            mark("sqrt_ln2(1)"); sqrt_ln2(1)
            mark("st7(1,0,4)"); st7(1, 0, 4)
            mark("st7(1,4,8)"); st7(1, 4, NT)
            mark("yout(1)"); yout(1)

    nc.compile()
    return nc


def _get_compiled():
    global _COMPILED
    if _COMPILED is None:
        _COMPILED = _build()
    return _COMPILED


def _prep_inputs(inputs):
    f32 = np.float32

    def a(name):
        return np.asarray(inputs[name], dtype=f32)

    x = a("x")
    mw = a("mother_wavelets")
    scales = a("scales")
    norm = np.sqrt(np.sum(mw ** 2, axis=2, keepdims=True))
    kern = (mw / np.maximum(norm, 1e-12)) * (1.0 / (1.0 + np.exp(-scales)))
    kern = kern[0, :, :, 0]                      # (W, H)
    kernT = np.ascontiguousarray(kern.T).astype(FP16)

    w1a = np.concatenate([a("mix_w1"), a("mix_b1")[None, :]], axis=0).astype(FP16)
    gln = np.ascontiguousarray(a("mix_ln_g").reshape(KM, 128).T).astype(f32)
    bln = np.ascontiguousarray(a("mix_ln_b").reshape(KM, 128).T).astype(f32)
    w2 = a("mix_w2").astype(FP16)
    b2c = np.ascontiguousarray(a("mix_b2").reshape(KH, 128).T).astype(f32)
    b2r = (b2c * SC_RES).astype(f32)
    gw = (a("out_ln_g")[:, None] * a("out_w")).astype(FP16)
    bw_vec = a("out_ln_b") @ a("out_w") + a("out_b")
    bw = np.tile(bw_vec[None, :], (128, 1)).astype(FP16)

    shared = {
        "kernt": kernT, "w1a": w1a, "gln": gln, "bln": bln, "w2": w2,
        "b2c": b2c, "b2r": b2r,
        "wq8": (a("wq") * SC_W).astype(FP8), "wk8": (a("wk") * SC_W).astype(FP8),
        "wv8": (a("wv") * SC_W).astype(FP8), "wo8": (a("wo") * SC_W).astype(FP8),
        "ident": np.eye(128, dtype=FP16),
        "gw": gw, "bw": bw,
    }

    xc = x.reshape(N_CHUNKS, CHUNK, H)
    xt_all = np.ascontiguousarray(xc.transpose(0, 2, 1)).astype(FP16)  # (16, H, CHUNK)
    in_maps = []
    for core in range(N_CORES):
        m = dict(shared)
        m["xt"] = np.ascontiguousarray(xt_all[core * CPC:(core + 1) * CPC])
        in_maps.append(m)
    return in_maps


def kernel(**inputs) -> np.ndarray:
    from concourse.bass_utils import run_bass_kernel_spmd

    nc = _get_compiled()
    in_maps = _prep_inputs(inputs)
    res = run_bass_kernel_spmd(nc, in_maps, core_ids=list(range(N_CORES)))
    out = np.concatenate([r["y"] for r in res.results], axis=0)  # (16, CHUNK, G)
    return out.reshape(B, S, G).astype(np.float32)
